# revision 13
# baseline (speedup 1.0000x reference)
"""Trainium2 Bass kernel for nn_DetectPeaksCC (NMS peak detection on xcorr).

Reference computation (per (nb, nc, nx) row of nt=4096 samples):
  x = |xcorr|; local-max mask (3-window); top-2 peak values s0,s1 + argmax i0;
  weight = (0.1 + 3(s0-s1)) s0^2; 3-point parabola through |x| at i0-1,i0,i0+1
  evaluated on a 201-point grid -> sub-sample shift + peak score; channel with
  max weight selected; outputs [max_cc, weight, shift_t, shift_idx].

Strategy (pure data-parallel over 8 cores, nb sharded 4 per core):
  - Host reorders each core's shard to rows r = c*256 + b*64 + x (channel
    outermost) and pads each 4096-row with one zero on each side -> [768, 4098].
  - Device: per 128-row tile, one DVE grouped reduce (abs-max over groups of
    16) -> [128, 256] group maxima; DVE max/max_index -> top-8 groups/row.
  - One batched indirect DMA gathers an 18-wide window (group + 1 neighbor on
    each side, from the padded rows so no clamping is needed) for every top
    group of every row.
  - All NMS / top-2 / parabola / channel-select logic runs on these small
    gathered tiles.  Group-16 windows have disjoint candidate sets so the
    top-2 peak extraction is exact as long as both peak-containing groups are
    within the per-row top-8 group maxima (verified against the reference).
"""

import sys

import numpy as np

if "/opt/trn_rl_repo" not in sys.path:
    sys.path.insert(0, "/opt/trn_rl_repo")

NB, NCH, NX, NT = 32, 3, 64, 4096
NCORES = 8
BPC = NB // NCORES            # batches per core
ROWS = NCH * BPC * NX         # 768 rows per core
RPAD = NT + 2                 # padded row length
P = 128
NTILES = ROWS // P            # 6
G = 16                        # group size along lag axis
NG = NT // G                  # 256 groups
K = 3                         # top groups drilled per row
WIN = G + 2                   # gathered window width
NGRID = 201
BIG = 1.0e9

_CACHE = {}


def _build_nc(debug_outputs=False):
    import concourse.bass as bass
    import concourse.tile as tile
    from concourse import mybir

    f32 = mybir.dt.float32
    i32 = mybir.dt.int32
    u32 = mybir.dt.uint32
    Alu = mybir.AluOpType
    Ax = mybir.AxisListType

    from concourse import bacc

    nc = bacc.Bacc("TRN2")

    xpad = nc.dram_tensor("xpad", [ROWS, RPAD], f32, kind="ExternalInput")
    rec = nc.dram_tensor("rec", [ROWS * NG, WIN], f32, kind="ExternalInput")
    xgd = nc.dram_tensor("xg", [1, NGRID], f32, kind="ExternalInput")
    nlagd = nc.dram_tensor("nlag_f", [P, 1], f32, kind="ExternalInput")
    outd = nc.dram_tensor("out", [4, 2 * P], f32, kind="ExternalOutput")

    from contextlib import ExitStack

    with tile.TileContext(nc) as tc, ExitStack() as ctx:
        const = ctx.enter_context(tc.tile_pool(name="const", bufs=1))
        xin = ctx.enter_context(tc.tile_pool(name="xin", bufs=NTILES))
        wk = ctx.enter_context(tc.tile_pool(name="wk", bufs=1))

        # ---- constants ----
        ramp_i = const.tile([P, WIN], i32)
        nc.gpsimd.iota(ramp_i[:], pattern=[[1, WIN]], base=-1, channel_multiplier=0)
        ramp = const.tile([P, WIN], f32)
        nc.vector.tensor_copy(ramp[:], ramp_i[:])  # -1..16 per partition

        rowb_i = const.tile([P, NTILES], i32)  # t*128+p
        nc.gpsimd.iota(
            rowb_i[:], pattern=[[P, NTILES]], base=0, channel_multiplier=1
        )
        rowb = const.tile([P, NTILES], f32)  # (t*128+p)*NG
        nc.vector.tensor_copy(rowb[:], rowb_i[:])
        nc.vector.tensor_scalar_mul(rowb[:], rowb[:], float(NG))

        xg = const.tile([P, NGRID], f32)
        nc.sync.dma_start(
            out=xg[:],
            in_=bass.AP(tensor=xgd, offset=0, ap=[[0, P], [1, NGRID]]),
        )
        xgp3 = const.tile([P, NGRID], f32)
        nc.vector.tensor_scalar_add(xgp3[:], xg[:], 3.0)

        nlag_t = const.tile([P, 1], f32)
        nc.sync.dma_start(out=nlag_t[:], in_=nlagd[:, :])

        # ---- phase 1: per-tile group abs-max + top-8 groups ----
        GM = wk.tile([P, NTILES * NG], f32)
        M8 = wk.tile([P, NTILES * 8], f32)
        MI = wk.tile([P, NTILES * 8], u32)
        for t in range(NTILES):
            Xt = xin.tile([P, NT], f32, tag="xt")
            nc.sync.dma_start(out=Xt[:], in_=xpad[t * P : (t + 1) * P, 1 : NT + 1])
            nc.vector.tensor_reduce(
                out=GM[:, t * NG : (t + 1) * NG],
                in_=Xt[:].rearrange("p (g e) -> p g e", e=G),
                axis=Ax.X,
                op=Alu.max,
                apply_absolute_value=True,
            )
            nc.vector.max(
                out=M8[:, t * 8 : (t + 1) * 8], in_=GM[:, t * NG : (t + 1) * NG]
            )
            nc.vector.max_index(
                out=MI[:, t * 8 : (t + 1) * 8],
                in_max=M8[:, t * 8 : (t + 1) * 8],
                in_values=GM[:, t * NG : (t + 1) * NG],
            )

        # s0 per (row-slot) = top-1 group max = global |x| max
        s0 = M8[:].rearrange("p (t k) -> p t k", k=8)[:, :, 0]  # [P, NTILES] view

        # ---- batched drill-down ----
        MIf = wk.tile([P, NTILES * K], f32)
        nc.vector.tensor_copy(
            MIf[:].rearrange("p (t k) -> p t k", k=K),
            MI[:].rearrange("p (t k) -> p t k", k=8)[:, :, 0:K],
        )

        # window positions in row coords: 16*g + (j-1), j=0..17
        POS = wk.tile([P, NTILES * K, WIN], f32)
        nc.vector.scalar_tensor_tensor(
            out=POS[:],
            in0=MIf[:].unsqueeze(2).to_broadcast([P, NTILES * K, WIN]),
            scalar=16.0,
            in1=ramp[:].unsqueeze(1).to_broadcast([P, NTILES * K, WIN]),
            op0=Alu.mult,
            op1=Alu.add,
        )

        # record indices into the window table: row*NG + g
        idxf = wk.tile([P, NTILES, K], f32)
        nc.vector.scalar_tensor_tensor(
            out=idxf[:],
            in0=MIf[:].rearrange("p (t k) -> p t k", k=K),
            scalar=1.0,
            in1=rowb[:].unsqueeze(2).to_broadcast([P, NTILES, K]),
            op0=Alu.mult,
            op1=Alu.add,
        )
        idxu = wk.tile([P, NTILES * K], u32)
        nc.vector.tensor_copy(idxu[:], idxf[:].rearrange("p t k -> p (t k)"))

        # [P,1]-offset embedding-style gathers (one per window slot) -- the
        # only indirect-DMA shape that works on HW.
        W = wk.tile([P, NTILES * K, WIN], f32)
        for sl in range(NTILES * K):
            nc.gpsimd.indirect_dma_start(
                out=W[:, sl, :],
                out_offset=None,
                in_=rec[:, :],
                in_offset=bass.IndirectOffsetOnAxis(ap=idxu[:, sl : sl + 1], axis=0),
            )
        AW = wk.tile([P, NTILES * K, WIN], f32)
        nc.scalar.activation(
            out=AW[:], in_=W[:], func=mybir.ActivationFunctionType.Abs
        )

        # NMS candidates: window positions 1..16 with in-window neighbors
        NBm = wk.tile([P, NTILES * K, G], f32)
        nc.vector.tensor_tensor(
            out=NBm[:], in0=AW[:, :, 0:G], in1=AW[:, :, 2 : G + 2], op=Alu.max
        )
        CM = wk.tile([P, NTILES * K, G], f32)
        nc.vector.tensor_tensor(
            out=CM[:], in0=AW[:, :, 1 : G + 1], in1=NBm[:], op=Alu.is_ge
        )
        CV = wk.tile([P, NTILES * K, G], f32)
        nc.vector.tensor_tensor(
            out=CV[:], in0=CM[:], in1=AW[:, :, 1 : G + 1], op=Alu.mult
        )

        CV4 = CV[:].rearrange("p (t k) m -> p t k m", k=K)
        POS4 = POS[:].rearrange("p (t k) j -> p t k j", k=K)

        # i0: first position in the top-group window where CV == s0
        eq0 = wk.tile([P, NTILES, G], f32)
        nc.vector.tensor_tensor(
            out=eq0[:],
            in0=CV4[:, :, 0, :],
            in1=s0.unsqueeze(2).to_broadcast([P, NTILES, G]),
            op=Alu.is_equal,
        )
        nm0 = wk.tile([P, NTILES, G], f32)
        nc.vector.tensor_scalar(
            nm0[:], eq0[:], 1.0, BIG, op0=Alu.not_equal, op1=Alu.mult
        )
        vpos = wk.tile([P, NTILES, G], f32)
        nc.vector.tensor_tensor(
            out=vpos[:], in0=nm0[:], in1=POS4[:, :, 0, 1 : G + 1], op=Alu.add
        )
        i0p = wk.tile([P, NTILES], f32)
        nc.vector.tensor_reduce(out=i0p[:], in_=vpos[:], axis=Ax.X, op=Alu.min)

        # s1: max candidate over all windows excluding position i0
        nem = wk.tile([P, NTILES * K, G], f32)
        nc.vector.tensor_tensor(
            out=nem[:].rearrange("p (t k) m -> p t k m", k=K),
            in0=POS4[:, :, :, 1 : G + 1],
            in1=i0p[:].unsqueeze(2).unsqueeze(3).to_broadcast([P, NTILES, K, G]),
            op=Alu.not_equal,
        )
        CV2 = wk.tile([P, NTILES * K, G], f32)
        nc.vector.tensor_tensor(out=CV2[:], in0=CV[:], in1=nem[:], op=Alu.mult)
        s1 = wk.tile([P, NTILES], f32)
        nc.vector.tensor_reduce(
            out=s1[:],
            in_=CV2[:].rearrange("p (t k) m -> p t k m", k=K),
            axis=Ax.XY,
            op=Alu.max,
        )

        # neighbors of i0 (from the top-group window, slot 0)
        tm1 = wk.tile([P, NTILES], f32)
        nc.vector.tensor_scalar(
            tm1[:], i0p[:], 1.0, 0.0, op0=Alu.subtract, op1=Alu.max
        )
        tp1 = wk.tile([P, NTILES], f32)
        nc.vector.tensor_scalar(
            tp1[:], i0p[:], 1.0, float(NT - 1), op0=Alu.add, op1=Alu.min
        )

        # results tile R[p, t, 0:5] = (weight, y0, ym1, yp1, i0)
        R = wk.tile([P, NTILES, 5], f32)

        for dst, sel in ((2, tm1), (3, tp1)):
            em = wk.tile([P, NTILES, WIN], f32, tag=f"em{dst}")
            nc.vector.tensor_tensor(
                out=em[:],
                in0=POS4[:, :, 0, :],
                in1=sel[:].unsqueeze(2).to_broadcast([P, NTILES, WIN]),
                op=Alu.is_equal,
            )
            pm = wk.tile([P, NTILES, WIN], f32, tag=f"pm{dst}")
            nc.vector.tensor_tensor(
                out=pm[:],
                in0=em[:],
                in1=AW[:].rearrange("p (t k) j -> p t k j", k=K)[:, :, 0, :],
                op=Alu.mult,
            )
            nc.vector.tensor_reduce(
                out=R[:, :, dst], in_=pm[:], axis=Ax.X, op=Alu.max
            )

        # weight = (0.1 + 3*(s0-s1)) * s0^2
        dd = wk.tile([P, NTILES], f32)
        nc.vector.tensor_tensor(out=dd[:], in0=s0, in1=s1[:], op=Alu.subtract)
        w1 = wk.tile([P, NTILES], f32)
        nc.vector.tensor_scalar(w1[:], dd[:], 3.0, 0.1, op0=Alu.mult, op1=Alu.add)
        s0sq = wk.tile([P, NTILES], f32)
        nc.vector.tensor_tensor(out=s0sq[:], in0=s0, in1=s0, op=Alu.mult)
        nc.vector.tensor_tensor(out=R[:, :, 0], in0=w1[:], in1=s0sq[:], op=Alu.mult)
        nc.vector.tensor_copy(R[:, :, 1], s0)
        nc.vector.tensor_copy(R[:, :, 4], i0p[:])

        # ---- channel combine: slot = c*2 + j ; argmax weight over c ----
        def exact_select(ga, on_true, on_false, name):
            # ga*on_true + (1-ga)*on_false: exact (one factor always 0, other 1)
            ngt = wk.tile([P, 2], f32, tag=f"ng_{name}")
            nc.vector.tensor_scalar(ngt[:], ga[:], 0.5, None, op0=Alu.is_lt)
            gb = ga[:].unsqueeze(2).to_broadcast([P, 2, 5])
            ngb = ngt[:].unsqueeze(2).to_broadcast([P, 2, 5])
            a1 = wk.tile([P, 2, 5], f32, tag=f"a1_{name}")
            nc.vector.tensor_tensor(out=a1[:], in0=on_true, in1=gb, op=Alu.mult)
            a2 = wk.tile([P, 2, 5], f32, tag=f"a2_{name}")
            nc.vector.tensor_tensor(out=a2[:], in0=on_false, in1=ngb, op=Alu.mult)
            res = wk.tile([P, 2, 5], f32, tag=f"res_{name}")
            nc.vector.tensor_tensor(out=res[:], in0=a1[:], in1=a2[:], op=Alu.add)
            return res

        g01 = wk.tile([P, 2], f32)
        nc.vector.tensor_tensor(
            out=g01[:], in0=R[:, 0:2, 0], in1=R[:, 2:4, 0], op=Alu.is_ge
        )
        B01 = exact_select(g01, R[:, 0:2, :], R[:, 2:4, :], "b01")
        g2 = wk.tile([P, 2], f32)
        nc.vector.tensor_tensor(
            out=g2[:], in0=B01[:, :, 0], in1=R[:, 4:6, 0], op=Alu.is_ge
        )
        FIN = exact_select(g2, B01[:], R[:, 4:6, :], "fin")

        # ---- parabola + grid argmax for the winning channel ----
        sm = wk.tile([P, 2], f32)
        nc.vector.tensor_tensor(
            out=sm[:], in0=FIN[:, :, 2], in1=FIN[:, :, 3], op=Alu.add
        )
        acf = wk.tile([P, 2], f32)
        nc.vector.scalar_tensor_tensor(
            out=acf[:],
            in0=sm[:],
            scalar=0.5,
            in1=FIN[:, :, 1],
            op0=Alu.mult,
            op1=Alu.subtract,
        )
        b2 = wk.tile([P, 2], f32)
        nc.vector.tensor_tensor(
            out=b2[:], in0=FIN[:, :, 3], in1=FIN[:, :, 2], op=Alu.subtract
        )
        bcf = wk.tile([P, 2], f32)
        nc.vector.tensor_scalar_mul(bcf[:], b2[:], 0.5)

        xgb = xg[:].unsqueeze(1).to_broadcast([P, 2, NGRID])
        t1 = wk.tile([P, 2, NGRID], f32)
        nc.vector.tensor_tensor(
            out=t1[:],
            in0=xgb,
            in1=acf[:].unsqueeze(2).to_broadcast([P, 2, NGRID]),
            op=Alu.mult,
        )
        nc.vector.tensor_tensor(
            out=t1[:],
            in0=t1[:],
            in1=bcf[:].unsqueeze(2).to_broadcast([P, 2, NGRID]),
            op=Alu.add,
        )
        yg = wk.tile([P, 2, NGRID], f32)
        nc.vector.tensor_tensor(out=yg[:], in0=t1[:], in1=xgb, op=Alu.mult)
        nc.vector.tensor_tensor(
            out=yg[:],
            in0=yg[:],
            in1=FIN[:, :, 1].unsqueeze(2).to_broadcast([P, 2, NGRID]),
            op=Alu.add,
        )

        O = wk.tile([P, 8], f32)  # [max_cc | w | shift_t | shift_idx] x (j0,j1)
        nc.vector.tensor_reduce(
            out=O[:, 0:2], in_=yg[:], axis=Ax.X, op=Alu.max
        )
        nmg = wk.tile([P, 2, NGRID], f32)
        nc.vector.tensor_tensor(
            out=nmg[:],
            in0=yg[:],
            in1=O[:, 0:2].unsqueeze(2).to_broadcast([P, 2, NGRID]),
            op=Alu.not_equal,
        )
        vg = wk.tile([P, 2, NGRID], f32)
        nc.vector.scalar_tensor_tensor(
            out=vg[:],
            in0=nmg[:],
            scalar=BIG,
            in1=xgp3[:].unsqueeze(1).to_broadcast([P, 2, NGRID]),
            op0=Alu.mult,
            op1=Alu.add,
        )
        sub3 = wk.tile([P, 2], f32)
        nc.vector.tensor_reduce(out=sub3[:], in_=vg[:], axis=Ax.X, op=Alu.min)

        nc.vector.tensor_copy(O[:, 2:4], FIN[:, :, 0])  # weight
        sub = wk.tile([P, 2], f32)
        nc.vector.tensor_scalar_sub(sub[:], sub3[:], 3.0)  # sub_shift
        idxw = wk.tile([P, 2], f32)
        nc.vector.tensor_tensor(
            out=idxw[:], in0=FIN[:, :, 4], in1=sub[:], op=Alu.add
        )
        nc.vector.tensor_tensor(
            out=O[:, 6:8],
            in0=idxw[:],
            in1=nlag_t[:].to_broadcast([P, 2]),
            op=Alu.subtract,
        )
        nc.vector.tensor_scalar_mul(O[:, 4:6], O[:, 6:8], 1.0 / 100.0)

        nc.sync.dma_start(
            out=outd[:, :].rearrange("m (j p) -> p m j", p=P),
            in_=O[:].rearrange("p (m j) -> p m j", j=2),
        )

        if debug_outputs:
            dumps = {
                "d_GM": (GM, NTILES * NG),
                "d_M8": (M8, NTILES * 8),
                "d_MI": (MI, NTILES * 8),
                "d_idx": (idxu, NTILES * K),
                "d_W": (W, NTILES * K * WIN),
                "d_AW": (AW, NTILES * K * WIN),
                "d_POS": (POS, NTILES * K * WIN),
                "d_CV": (CV, NTILES * K * G),
                "d_i0p": (i0p, NTILES),
                "d_s1": (s1, NTILES),
                "d_R": (R, NTILES * 5),
                "d_FIN": (FIN, 10),
                "d_sub3": (sub3, 2),
            }
            for name, (tl, fsz) in dumps.items():
                dt_ = tl[:].dtype
                dd = nc.dram_tensor(name, [P, fsz], dt_, kind="ExternalOutput")
                nc.sync.dma_start(
                    out=dd[:, :],
                    in_=tl[:].rearrange("p ... -> p (...)")
                    if tl[:].ndim > 2
                    else tl[:],
                )

    nc.finalize()
    return nc


def _get_nc():
    if "nc" not in _CACHE:
        _CACHE["nc"] = _build_nc()
    return _CACHE["nc"]


def _xg_host():
    import jax
    import jax.numpy as jnp

    with jax.default_device(jax.devices("cpu")[0]):
        return np.asarray(jnp.linspace(-1.0, 1.0, NGRID, dtype=jnp.float32))


def shard_inputs(xcorr, nlag):
    """Full [32,3,64,4096] -> list of 8 per-core input maps."""
    xcorr = np.asarray(xcorr, dtype=np.float32)
    xg = _xg_host()
    nlag_f = np.full([P, 1], float(int(nlag)), dtype=np.float32)
    in_maps = []
    for k in range(NCORES):
        sh = xcorr[k * BPC : (k + 1) * BPC]          # [4, 3, 64, 4096]
        sh = np.ascontiguousarray(sh.transpose(1, 0, 2, 3)).reshape(ROWS, NT)
        pad = np.zeros([ROWS, RPAD], dtype=np.float32)
        pad[:, 1 : NT + 1] = sh
        # window-record table: rec[r*NG + g, :] = pad[r, 16g : 16g+18]
        recs = np.lib.stride_tricks.sliding_window_view(pad, WIN, axis=1)[:, ::G, :]
        recs = np.ascontiguousarray(recs).reshape(ROWS * NG, WIN)
        in_maps.append(
            {
                "xpad": pad,
                "rec": recs,
                "xg": xg.reshape(1, NGRID).copy(),
                "nlag_f": nlag_f.copy(),
            }
        )
    return in_maps


def unshard_outputs(results):
    """list of 8 per-core {'out': [4,256]} -> [4, 32, 1, 64]."""
    full = np.zeros([4, NB, 1, NX], dtype=np.float32)
    for k, res in enumerate(results):
        o = np.asarray(res["out"], dtype=np.float32).reshape(4, BPC, NX)
        full[:, k * BPC : (k + 1) * BPC, 0, :] = o
    return full


def kernel(xcorr, nlag):
    from concourse.bass_utils import run_bass_kernel_spmd

    nc = _get_nc()
    in_maps = shard_inputs(xcorr, nlag)
    res = run_bass_kernel_spmd(nc, in_maps, list(range(NCORES)))
    return unshard_outputs(res.results)


# revision 14
# speedup vs baseline: 1.2033x; 1.2033x over previous
"""Trainium2 Bass kernel for nn_DetectPeaksCC (NMS peak detection on xcorr).

Reference computation (per (nb, nc, nx) row of nt=4096 samples):
  x = |xcorr|; local-max mask (3-window); top-2 peak values s0,s1 + argmax i0;
  weight = (0.1 + 3(s0-s1)) s0^2; 3-point parabola through |x| at i0-1,i0,i0+1
  evaluated on a 201-point grid -> sub-sample shift + peak score; channel with
  max weight selected; outputs [max_cc, weight, shift_t, shift_idx].

Strategy (pure data-parallel over 8 cores, nb sharded 4 per core):
  - Host reorders each core's shard to rows r = c*256 + b*64 + x (channel
    outermost) and pads each 4096-row with one zero on each side -> [768, 4098].
  - Device: per 128-row tile, one DVE grouped reduce (abs-max over groups of
    16) -> [128, 256] group maxima; DVE max/max_index -> top-8 groups/row.
  - One batched indirect DMA gathers an 18-wide window (group + 1 neighbor on
    each side, from the padded rows so no clamping is needed) for every top
    group of every row.
  - All NMS / top-2 / parabola / channel-select logic runs on these small
    gathered tiles.  Group-16 windows have disjoint candidate sets so the
    top-2 peak extraction is exact as long as both peak-containing groups are
    within the per-row top-8 group maxima (verified against the reference).
"""

import sys

import numpy as np

if "/opt/trn_rl_repo" not in sys.path:
    sys.path.insert(0, "/opt/trn_rl_repo")

NB, NCH, NX, NT = 32, 3, 64, 4096
NCORES = 8
BPC = NB // NCORES            # batches per core
ROWS = NCH * BPC * NX         # 768 rows per core
RPAD = NT + 2                 # padded row length
P = 128
NTILES = ROWS // P            # 6
G = 16                        # group size along lag axis
NG = NT // G                  # 256 groups
K = 3                         # top groups drilled per row
WIN = G + 2                   # gathered window width
NGRID = 201
BIG = 1.0e9

_CACHE = {}


def _build_nc(debug_outputs=False):
    import concourse.bass as bass
    import concourse.tile as tile
    from concourse import mybir

    f32 = mybir.dt.float32
    i32 = mybir.dt.int32
    u32 = mybir.dt.uint32
    Alu = mybir.AluOpType
    Ax = mybir.AxisListType

    from concourse import bacc

    nc = bacc.Bacc("TRN2")

    xpad = nc.dram_tensor("xpad", [ROWS, RPAD], f32, kind="ExternalInput")
    rec = nc.dram_tensor("rec", [ROWS * NG, WIN], f32, kind="ExternalInput")
    xgd = nc.dram_tensor("xg", [1, NGRID], f32, kind="ExternalInput")
    nlagd = nc.dram_tensor("nlag_f", [P, 1], f32, kind="ExternalInput")
    outd = nc.dram_tensor("out", [4, 2 * P], f32, kind="ExternalOutput")

    from contextlib import ExitStack

    with tile.TileContext(nc) as tc, ExitStack() as ctx:
        const = ctx.enter_context(tc.tile_pool(name="const", bufs=1))
        xin = ctx.enter_context(tc.tile_pool(name="xin", bufs=NTILES))
        wk = ctx.enter_context(tc.tile_pool(name="wk", bufs=1))

        # ---- constants ----
        ramp_i = const.tile([P, WIN], i32)
        nc.gpsimd.iota(ramp_i[:], pattern=[[1, WIN]], base=-1, channel_multiplier=0)
        ramp = const.tile([P, WIN], f32)
        nc.vector.tensor_copy(ramp[:], ramp_i[:])  # -1..16 per partition

        rowb_i = const.tile([P, NTILES], i32)  # t*128+p
        nc.gpsimd.iota(
            rowb_i[:], pattern=[[P, NTILES]], base=0, channel_multiplier=1
        )
        rowb = const.tile([P, NTILES], f32)  # (t*128+p)*NG
        nc.vector.tensor_copy(rowb[:], rowb_i[:])
        nc.vector.tensor_scalar_mul(rowb[:], rowb[:], float(NG))

        xg = const.tile([P, NGRID], f32)
        nc.sync.dma_start(
            out=xg[:],
            in_=bass.AP(tensor=xgd, offset=0, ap=[[0, P], [1, NGRID]]),
        )
        xgp3 = const.tile([P, NGRID], f32)
        nc.vector.tensor_scalar_add(xgp3[:], xg[:], 3.0)

        nlag_t = const.tile([P, 1], f32)
        nc.sync.dma_start(out=nlag_t[:], in_=nlagd[:, :])
        # warm the ACT Abs/Identity table set early so the table load is off
        # the critical path
        warm = const.tile([P, 1], f32)
        nc.scalar.activation(
            out=warm[:], in_=nlag_t[:], func=mybir.ActivationFunctionType.Abs
        )

        # ---- phase 1: per-tile group abs-max + top-8 groups + window gathers
        GM = wk.tile([P, NTILES * NG], f32)
        M8 = wk.tile([P, NTILES * 8], f32)
        MI = wk.tile([P, NTILES * 8], u32)
        POS = wk.tile([P, NTILES * K, WIN], f32)
        idxu = wk.tile([P, NTILES * K], u32)
        idxf = wk.tile([P, NTILES, K], f32)
        W = wk.tile([P, NTILES * K, WIN], f32)
        for t in range(NTILES):
            Xt = xin.tile([P, NT], f32, tag="xt")
            dma_eng = nc.sync if t % 2 == 0 else nc.scalar
            dma_eng.dma_start(out=Xt[:], in_=xpad[t * P : (t + 1) * P, 1 : NT + 1])
            nc.vector.tensor_reduce(
                out=GM[:, t * NG : (t + 1) * NG],
                in_=Xt[:].rearrange("p (g e) -> p g e", e=G),
                axis=Ax.X,
                op=Alu.max,
                apply_absolute_value=True,
            )
            nc.vector.max(
                out=M8[:, t * 8 : (t + 1) * 8], in_=GM[:, t * NG : (t + 1) * NG]
            )
            nc.vector.max_index(
                out=MI[:, t * 8 : (t + 1) * 8],
                in_max=M8[:, t * 8 : (t + 1) * 8],
                in_values=GM[:, t * NG : (t + 1) * NG],
            )
            MI_t = MI[:].rearrange("p (t k) -> p t k", k=8)[:, t, 0:K]  # [P, K] u32
            # window positions in row coords: 16*g + (j-1), j=0..17
            nc.vector.scalar_tensor_tensor(
                out=POS[:, t * K : (t + 1) * K, :],
                in0=MI_t.unsqueeze(2).to_broadcast([P, K, WIN]),
                scalar=16.0,
                in1=ramp[:].unsqueeze(1).to_broadcast([P, K, WIN]),
                op0=Alu.mult,
                op1=Alu.add,
            )
            # record indices into the window table: row*NG + g
            nc.vector.scalar_tensor_tensor(
                out=idxf[:, t, :],
                in0=MI_t,
                scalar=1.0,
                in1=rowb[:, t : t + 1].to_broadcast([P, K]),
                op0=Alu.mult,
                op1=Alu.add,
            )
            nc.vector.tensor_copy(idxu[:, t * K : (t + 1) * K], idxf[:, t, :])
            # [P,1]-offset embedding-style gathers (one per window slot) --
            # the only indirect-DMA shape that works on HW.
            for k in range(K):
                sl = t * K + k
                nc.gpsimd.indirect_dma_start(
                    out=W[:, sl, :],
                    out_offset=None,
                    in_=rec[:, :],
                    in_offset=bass.IndirectOffsetOnAxis(
                        ap=idxu[:, sl : sl + 1], axis=0
                    ),
                )

        # s0 per (row-slot) = top-1 group max = global |x| max
        s0 = M8[:].rearrange("p (t k) -> p t k", k=8)[:, :, 0]  # [P, NTILES] view
        AW = wk.tile([P, NTILES * K, WIN], f32)
        nc.scalar.activation(
            out=AW[:], in_=W[:], func=mybir.ActivationFunctionType.Abs
        )

        # NMS candidates: window positions 1..16 with in-window neighbors
        NBm = wk.tile([P, NTILES * K, G], f32)
        nc.vector.tensor_tensor(
            out=NBm[:], in0=AW[:, :, 0:G], in1=AW[:, :, 2 : G + 2], op=Alu.max
        )
        CM = wk.tile([P, NTILES * K, G], f32)
        nc.vector.tensor_tensor(
            out=CM[:], in0=AW[:, :, 1 : G + 1], in1=NBm[:], op=Alu.is_ge
        )
        CV = wk.tile([P, NTILES * K, G], f32)
        nc.vector.tensor_tensor(
            out=CV[:], in0=CM[:], in1=AW[:, :, 1 : G + 1], op=Alu.mult
        )

        CV4 = CV[:].rearrange("p (t k) m -> p t k m", k=K)
        POS4 = POS[:].rearrange("p (t k) j -> p t k j", k=K)

        # i0: first position in the top-group window where CV == s0
        eq0 = wk.tile([P, NTILES, G], f32)
        nc.vector.tensor_tensor(
            out=eq0[:],
            in0=CV4[:, :, 0, :],
            in1=s0.unsqueeze(2).to_broadcast([P, NTILES, G]),
            op=Alu.is_equal,
        )
        nm0 = wk.tile([P, NTILES, G], f32)
        nc.vector.tensor_scalar(
            nm0[:], eq0[:], 1.0, BIG, op0=Alu.not_equal, op1=Alu.mult
        )
        vpos = wk.tile([P, NTILES, G], f32)
        nc.vector.tensor_tensor(
            out=vpos[:], in0=nm0[:], in1=POS4[:, :, 0, 1 : G + 1], op=Alu.add
        )
        i0p = wk.tile([P, NTILES], f32)
        nc.vector.tensor_reduce(out=i0p[:], in_=vpos[:], axis=Ax.X, op=Alu.min)

        # s1: max candidate over all windows excluding position i0
        nem = wk.tile([P, NTILES * K, G], f32)
        nc.vector.tensor_tensor(
            out=nem[:].rearrange("p (t k) m -> p t k m", k=K),
            in0=POS4[:, :, :, 1 : G + 1],
            in1=i0p[:].unsqueeze(2).unsqueeze(3).to_broadcast([P, NTILES, K, G]),
            op=Alu.not_equal,
        )
        CV2 = wk.tile([P, NTILES * K, G], f32)
        nc.vector.tensor_tensor(out=CV2[:], in0=CV[:], in1=nem[:], op=Alu.mult)
        s1 = wk.tile([P, NTILES], f32)
        nc.vector.tensor_reduce(
            out=s1[:],
            in_=CV2[:].rearrange("p (t k) m -> p t k m", k=K),
            axis=Ax.XY,
            op=Alu.max,
        )

        # neighbors of i0 (from the top-group window, slot 0)
        tm1 = wk.tile([P, NTILES], f32)
        nc.vector.tensor_scalar(
            tm1[:], i0p[:], 1.0, 0.0, op0=Alu.subtract, op1=Alu.max
        )
        tp1 = wk.tile([P, NTILES], f32)
        nc.vector.tensor_scalar(
            tp1[:], i0p[:], 1.0, float(NT - 1), op0=Alu.add, op1=Alu.min
        )

        # results tile R[p, t, 0:5] = (weight, y0, ym1, yp1, i0)
        R = wk.tile([P, NTILES, 5], f32)

        for dst, sel in ((2, tm1), (3, tp1)):
            em = wk.tile([P, NTILES, WIN], f32, tag=f"em{dst}")
            nc.vector.tensor_tensor(
                out=em[:],
                in0=POS4[:, :, 0, :],
                in1=sel[:].unsqueeze(2).to_broadcast([P, NTILES, WIN]),
                op=Alu.is_equal,
            )
            pm = wk.tile([P, NTILES, WIN], f32, tag=f"pm{dst}")
            nc.vector.tensor_tensor(
                out=pm[:],
                in0=em[:],
                in1=AW[:].rearrange("p (t k) j -> p t k j", k=K)[:, :, 0, :],
                op=Alu.mult,
            )
            nc.vector.tensor_reduce(
                out=R[:, :, dst], in_=pm[:], axis=Ax.X, op=Alu.max
            )

        # weight = (0.1 + 3*(s0-s1)) * s0^2
        dd = wk.tile([P, NTILES], f32)
        nc.vector.tensor_tensor(out=dd[:], in0=s0, in1=s1[:], op=Alu.subtract)
        w1 = wk.tile([P, NTILES], f32)
        nc.vector.tensor_scalar(w1[:], dd[:], 3.0, 0.1, op0=Alu.mult, op1=Alu.add)
        s0sq = wk.tile([P, NTILES], f32)
        nc.vector.tensor_tensor(out=s0sq[:], in0=s0, in1=s0, op=Alu.mult)
        nc.vector.tensor_tensor(out=R[:, :, 0], in0=w1[:], in1=s0sq[:], op=Alu.mult)
        nc.vector.tensor_copy(R[:, :, 1], s0)
        nc.vector.tensor_copy(R[:, :, 4], i0p[:])

        # ---- channel combine: slot = c*2 + j ; argmax weight over c ----
        def exact_select(ga, on_true, on_false, name):
            # ga*on_true + (1-ga)*on_false: exact (one factor always 0, other 1)
            ngt = wk.tile([P, 2], f32, tag=f"ng_{name}")
            nc.vector.tensor_scalar(ngt[:], ga[:], 0.5, None, op0=Alu.is_lt)
            gb = ga[:].unsqueeze(2).to_broadcast([P, 2, 5])
            ngb = ngt[:].unsqueeze(2).to_broadcast([P, 2, 5])
            a1 = wk.tile([P, 2, 5], f32, tag=f"a1_{name}")
            nc.vector.tensor_tensor(out=a1[:], in0=on_true, in1=gb, op=Alu.mult)
            a2 = wk.tile([P, 2, 5], f32, tag=f"a2_{name}")
            nc.vector.tensor_tensor(out=a2[:], in0=on_false, in1=ngb, op=Alu.mult)
            res = wk.tile([P, 2, 5], f32, tag=f"res_{name}")
            nc.vector.tensor_tensor(out=res[:], in0=a1[:], in1=a2[:], op=Alu.add)
            return res

        g01 = wk.tile([P, 2], f32)
        nc.vector.tensor_tensor(
            out=g01[:], in0=R[:, 0:2, 0], in1=R[:, 2:4, 0], op=Alu.is_ge
        )
        B01 = exact_select(g01, R[:, 0:2, :], R[:, 2:4, :], "b01")
        g2 = wk.tile([P, 2], f32)
        nc.vector.tensor_tensor(
            out=g2[:], in0=B01[:, :, 0], in1=R[:, 4:6, 0], op=Alu.is_ge
        )
        FIN = exact_select(g2, B01[:], R[:, 4:6, :], "fin")

        # ---- parabola + grid argmax for the winning channel ----
        sm = wk.tile([P, 2], f32)
        nc.vector.tensor_tensor(
            out=sm[:], in0=FIN[:, :, 2], in1=FIN[:, :, 3], op=Alu.add
        )
        acf = wk.tile([P, 2], f32)
        nc.vector.scalar_tensor_tensor(
            out=acf[:],
            in0=sm[:],
            scalar=0.5,
            in1=FIN[:, :, 1],
            op0=Alu.mult,
            op1=Alu.subtract,
        )
        b2 = wk.tile([P, 2], f32)
        nc.vector.tensor_tensor(
            out=b2[:], in0=FIN[:, :, 3], in1=FIN[:, :, 2], op=Alu.subtract
        )
        bcf = wk.tile([P, 2], f32)
        nc.vector.tensor_scalar_mul(bcf[:], b2[:], 0.5)

        xgb = xg[:].unsqueeze(1).to_broadcast([P, 2, NGRID])
        t1 = wk.tile([P, 2, NGRID], f32)
        nc.vector.tensor_tensor(
            out=t1[:],
            in0=xgb,
            in1=acf[:].unsqueeze(2).to_broadcast([P, 2, NGRID]),
            op=Alu.mult,
        )
        nc.vector.tensor_tensor(
            out=t1[:],
            in0=t1[:],
            in1=bcf[:].unsqueeze(2).to_broadcast([P, 2, NGRID]),
            op=Alu.add,
        )
        yg = wk.tile([P, 2, NGRID], f32)
        nc.vector.tensor_tensor(out=yg[:], in0=t1[:], in1=xgb, op=Alu.mult)
        nc.vector.tensor_tensor(
            out=yg[:],
            in0=yg[:],
            in1=FIN[:, :, 1].unsqueeze(2).to_broadcast([P, 2, NGRID]),
            op=Alu.add,
        )

        O = wk.tile([P, 8], f32)  # [max_cc | w | shift_t | shift_idx] x (j0,j1)
        nc.vector.tensor_reduce(
            out=O[:, 0:2], in_=yg[:], axis=Ax.X, op=Alu.max
        )
        nmg = wk.tile([P, 2, NGRID], f32)
        nc.vector.tensor_tensor(
            out=nmg[:],
            in0=yg[:],
            in1=O[:, 0:2].unsqueeze(2).to_broadcast([P, 2, NGRID]),
            op=Alu.not_equal,
        )
        vg = wk.tile([P, 2, NGRID], f32)
        nc.vector.scalar_tensor_tensor(
            out=vg[:],
            in0=nmg[:],
            scalar=BIG,
            in1=xgp3[:].unsqueeze(1).to_broadcast([P, 2, NGRID]),
            op0=Alu.mult,
            op1=Alu.add,
        )
        sub3 = wk.tile([P, 2], f32)
        nc.vector.tensor_reduce(out=sub3[:], in_=vg[:], axis=Ax.X, op=Alu.min)

        nc.vector.tensor_copy(O[:, 2:4], FIN[:, :, 0])  # weight
        sub = wk.tile([P, 2], f32)
        nc.vector.tensor_scalar_sub(sub[:], sub3[:], 3.0)  # sub_shift
        idxw = wk.tile([P, 2], f32)
        nc.vector.tensor_tensor(
            out=idxw[:], in0=FIN[:, :, 4], in1=sub[:], op=Alu.add
        )
        nc.vector.tensor_tensor(
            out=O[:, 6:8],
            in0=idxw[:],
            in1=nlag_t[:].to_broadcast([P, 2]),
            op=Alu.subtract,
        )
        nc.vector.tensor_scalar_mul(O[:, 4:6], O[:, 6:8], 1.0 / 100.0)

        nc.sync.dma_start(
            out=outd[:, :].rearrange("m (j p) -> p m j", p=P),
            in_=O[:].rearrange("p (m j) -> p m j", j=2),
        )

        if debug_outputs:
            dumps = {
                "d_GM": (GM, NTILES * NG),
                "d_M8": (M8, NTILES * 8),
                "d_MI": (MI, NTILES * 8),
                "d_idx": (idxu, NTILES * K),
                "d_W": (W, NTILES * K * WIN),
                "d_AW": (AW, NTILES * K * WIN),
                "d_POS": (POS, NTILES * K * WIN),
                "d_CV": (CV, NTILES * K * G),
                "d_i0p": (i0p, NTILES),
                "d_s1": (s1, NTILES),
                "d_R": (R, NTILES * 5),
                "d_FIN": (FIN, 10),
                "d_sub3": (sub3, 2),
            }
            for name, (tl, fsz) in dumps.items():
                dt_ = tl[:].dtype
                dd = nc.dram_tensor(name, [P, fsz], dt_, kind="ExternalOutput")
                nc.sync.dma_start(
                    out=dd[:, :],
                    in_=tl[:].rearrange("p ... -> p (...)")
                    if tl[:].ndim > 2
                    else tl[:],
                )

    nc.finalize()
    return nc


def _get_nc():
    if "nc" not in _CACHE:
        _CACHE["nc"] = _build_nc()
    return _CACHE["nc"]


def _xg_host():
    import jax
    import jax.numpy as jnp

    with jax.default_device(jax.devices("cpu")[0]):
        return np.asarray(jnp.linspace(-1.0, 1.0, NGRID, dtype=jnp.float32))


def shard_inputs(xcorr, nlag):
    """Full [32,3,64,4096] -> list of 8 per-core input maps."""
    xcorr = np.asarray(xcorr, dtype=np.float32)
    xg = _xg_host()
    nlag_f = np.full([P, 1], float(int(nlag)), dtype=np.float32)
    in_maps = []
    for k in range(NCORES):
        sh = xcorr[k * BPC : (k + 1) * BPC]          # [4, 3, 64, 4096]
        sh = np.ascontiguousarray(sh.transpose(1, 0, 2, 3)).reshape(ROWS, NT)
        pad = np.zeros([ROWS, RPAD], dtype=np.float32)
        pad[:, 1 : NT + 1] = sh
        # window-record table: rec[r*NG + g, :] = pad[r, 16g : 16g+18]
        recs = np.lib.stride_tricks.sliding_window_view(pad, WIN, axis=1)[:, ::G, :]
        recs = np.ascontiguousarray(recs).reshape(ROWS * NG, WIN)
        in_maps.append(
            {
                "xpad": pad,
                "rec": recs,
                "xg": xg.reshape(1, NGRID).copy(),
                "nlag_f": nlag_f.copy(),
            }
        )
    return in_maps


def unshard_outputs(results):
    """list of 8 per-core {'out': [4,256]} -> [4, 32, 1, 64]."""
    full = np.zeros([4, NB, 1, NX], dtype=np.float32)
    for k, res in enumerate(results):
        o = np.asarray(res["out"], dtype=np.float32).reshape(4, BPC, NX)
        full[:, k * BPC : (k + 1) * BPC, 0, :] = o
    return full


def kernel(xcorr, nlag):
    from concourse.bass_utils import run_bass_kernel_spmd

    nc = _get_nc()
    in_maps = shard_inputs(xcorr, nlag)
    res = run_bass_kernel_spmd(nc, in_maps, list(range(NCORES)))
    return unshard_outputs(res.results)


# revision 15
# speedup vs baseline: 1.2126x; 1.0077x over previous
"""Trainium2 Bass kernel for nn_DetectPeaksCC (NMS peak detection on xcorr).

Reference computation (per (nb, nc, nx) row of nt=4096 samples):
  x = |xcorr|; local-max mask (3-window); top-2 peak values s0,s1 + argmax i0;
  weight = (0.1 + 3(s0-s1)) s0^2; 3-point parabola through |x| at i0-1,i0,i0+1
  evaluated on a 201-point grid -> sub-sample shift + peak score; channel with
  max weight selected; outputs [max_cc, weight, shift_t, shift_idx].

Strategy (pure data-parallel over 8 cores, nb sharded 4 per core):
  - Host reorders each core's shard to rows r = c*256 + b*64 + x (channel
    outermost) and pads each 4096-row with one zero on each side -> [768, 4098].
  - Device: per 128-row tile, one DVE grouped reduce (abs-max over groups of
    16) -> [128, 256] group maxima; DVE max/max_index -> top-8 groups/row.
  - One batched indirect DMA gathers an 18-wide window (group + 1 neighbor on
    each side, from the padded rows so no clamping is needed) for every top
    group of every row.
  - All NMS / top-2 / parabola / channel-select logic runs on these small
    gathered tiles.  Group-16 windows have disjoint candidate sets so the
    top-2 peak extraction is exact as long as both peak-containing groups are
    within the per-row top-8 group maxima (verified against the reference).
"""

import sys

import numpy as np

if "/opt/trn_rl_repo" not in sys.path:
    sys.path.insert(0, "/opt/trn_rl_repo")

NB, NCH, NX, NT = 32, 3, 64, 4096
NCORES = 8
BPC = NB // NCORES            # batches per core
ROWS = NCH * BPC * NX         # 768 rows per core
RPAD = NT + 2                 # padded row length
P = 128
NTILES = ROWS // P            # 6
G = 16                        # group size along lag axis
NG = NT // G                  # 256 groups
K = 2                         # top groups drilled per row
WIN = G + 2                   # gathered window width
NGRID = 201
BIG = 1.0e9

_CACHE = {}


def _build_nc(debug_outputs=False):
    import concourse.bass as bass
    import concourse.tile as tile
    from concourse import mybir

    f32 = mybir.dt.float32
    i32 = mybir.dt.int32
    u32 = mybir.dt.uint32
    Alu = mybir.AluOpType
    Ax = mybir.AxisListType

    from concourse import bacc

    nc = bacc.Bacc("TRN2")

    xpad = nc.dram_tensor("xpad", [ROWS, RPAD], f32, kind="ExternalInput")
    rec = nc.dram_tensor("rec", [ROWS * NG, WIN], f32, kind="ExternalInput")
    xgd = nc.dram_tensor("xg", [1, NGRID], f32, kind="ExternalInput")
    nlagd = nc.dram_tensor("nlag_f", [P, 1], f32, kind="ExternalInput")
    outd = nc.dram_tensor("out", [4, 2 * P], f32, kind="ExternalOutput")

    from contextlib import ExitStack

    with tile.TileContext(nc) as tc, ExitStack() as ctx:
        const = ctx.enter_context(tc.tile_pool(name="const", bufs=1))
        xin = ctx.enter_context(tc.tile_pool(name="xin", bufs=NTILES))
        wk = ctx.enter_context(tc.tile_pool(name="wk", bufs=1))

        # ---- constants ----
        ramp_i = const.tile([P, WIN], i32)
        nc.gpsimd.iota(ramp_i[:], pattern=[[1, WIN]], base=-1, channel_multiplier=0)
        ramp = const.tile([P, WIN], f32)
        nc.vector.tensor_copy(ramp[:], ramp_i[:])  # -1..16 per partition

        rowb_i = const.tile([P, NTILES], i32)  # t*128+p
        nc.gpsimd.iota(
            rowb_i[:], pattern=[[P, NTILES]], base=0, channel_multiplier=1
        )
        rowb = const.tile([P, NTILES], f32)  # (t*128+p)*NG
        nc.vector.tensor_copy(rowb[:], rowb_i[:])
        nc.vector.tensor_scalar_mul(rowb[:], rowb[:], float(NG))

        xg = const.tile([P, NGRID], f32)
        nc.gpsimd.dma_start(
            out=xg[:],
            in_=bass.AP(tensor=xgd, offset=0, ap=[[0, P], [1, NGRID]]),
        )
        xgp3 = const.tile([P, NGRID], f32)
        nc.vector.tensor_scalar_add(xgp3[:], xg[:], 3.0)

        nlag_t = const.tile([P, 1], f32)
        nc.gpsimd.dma_start(out=nlag_t[:], in_=nlagd[:, :])
        # warm the ACT Abs/Identity table set early so the table load is off
        # the critical path
        warm = const.tile([P, 1], f32)
        nc.scalar.activation(
            out=warm[:], in_=nlag_t[:], func=mybir.ActivationFunctionType.Abs
        )

        # ---- phase 1: per-tile group abs-max + top-8 groups + window gathers
        GM = wk.tile([P, NTILES * NG], f32)
        M8 = wk.tile([P, NTILES * 8], f32)
        MI = wk.tile([P, NTILES * 8], u32)
        POS = wk.tile([P, NTILES * K, WIN], f32)
        idxu = wk.tile([P, NTILES * K], u32)
        idxf = wk.tile([P, NTILES, K], f32)
        W = wk.tile([P, NTILES * K, WIN], f32)
        HNT = NT // 2
        HNG = NG // 2
        for t in range(NTILES):
            Xt = xin.tile([P, NT], f32, tag="xt")
            for h in range(2):
                dma_eng = nc.sync if h == 0 else nc.scalar
                dma_eng.dma_start(
                    out=Xt[:, h * HNT : (h + 1) * HNT],
                    in_=xpad[t * P : (t + 1) * P, 1 + h * HNT : 1 + (h + 1) * HNT],
                )
                nc.vector.tensor_reduce(
                    out=GM[:, t * NG + h * HNG : t * NG + (h + 1) * HNG],
                    in_=Xt[:, h * HNT : (h + 1) * HNT].rearrange(
                        "p (g e) -> p g e", e=G
                    ),
                    axis=Ax.X,
                    op=Alu.max,
                    apply_absolute_value=True,
                )
            nc.vector.max(
                out=M8[:, t * 8 : (t + 1) * 8], in_=GM[:, t * NG : (t + 1) * NG]
            )
            nc.vector.max_index(
                out=MI[:, t * 8 : (t + 1) * 8],
                in_max=M8[:, t * 8 : (t + 1) * 8],
                in_values=GM[:, t * NG : (t + 1) * NG],
            )
            MI_t = MI[:].rearrange("p (t k) -> p t k", k=8)[:, t, 0:K]  # [P, K] u32
            # window positions in row coords: 16*g + (j-1), j=0..17
            nc.vector.scalar_tensor_tensor(
                out=POS[:, t * K : (t + 1) * K, :],
                in0=MI_t.unsqueeze(2).to_broadcast([P, K, WIN]),
                scalar=16.0,
                in1=ramp[:].unsqueeze(1).to_broadcast([P, K, WIN]),
                op0=Alu.mult,
                op1=Alu.add,
            )
            # record indices into the window table: row*NG + g
            nc.vector.scalar_tensor_tensor(
                out=idxf[:, t, :],
                in0=MI_t,
                scalar=1.0,
                in1=rowb[:, t : t + 1].to_broadcast([P, K]),
                op0=Alu.mult,
                op1=Alu.add,
            )
            nc.vector.tensor_copy(idxu[:, t * K : (t + 1) * K], idxf[:, t, :])
            # [P,1]-offset embedding-style gathers (one per window slot) --
            # the only indirect-DMA shape that works on HW.
            for k in range(K):
                sl = t * K + k
                nc.gpsimd.indirect_dma_start(
                    out=W[:, sl, :],
                    out_offset=None,
                    in_=rec[:, :],
                    in_offset=bass.IndirectOffsetOnAxis(
                        ap=idxu[:, sl : sl + 1], axis=0
                    ),
                )

        # s0 per (row-slot) = top-1 group max = global |x| max
        s0 = M8[:].rearrange("p (t k) -> p t k", k=8)[:, :, 0]  # [P, NTILES] view
        AW = wk.tile([P, NTILES * K, WIN], f32)
        nc.scalar.activation(
            out=AW[:], in_=W[:], func=mybir.ActivationFunctionType.Abs
        )

        # NMS candidates: window positions 1..16 with in-window neighbors
        NBm = wk.tile([P, NTILES * K, G], f32)
        nc.vector.tensor_tensor(
            out=NBm[:], in0=AW[:, :, 0:G], in1=AW[:, :, 2 : G + 2], op=Alu.max
        )
        CM = wk.tile([P, NTILES * K, G], f32)
        nc.vector.tensor_tensor(
            out=CM[:], in0=AW[:, :, 1 : G + 1], in1=NBm[:], op=Alu.is_ge
        )
        CV = wk.tile([P, NTILES * K, G], f32)
        nc.vector.tensor_tensor(
            out=CV[:], in0=CM[:], in1=AW[:, :, 1 : G + 1], op=Alu.mult
        )

        CV4 = CV[:].rearrange("p (t k) m -> p t k m", k=K)
        POS4 = POS[:].rearrange("p (t k) j -> p t k j", k=K)

        # i0: first position in the top-group window where CV == s0
        eq0 = wk.tile([P, NTILES, G], f32)
        nc.vector.tensor_tensor(
            out=eq0[:],
            in0=CV4[:, :, 0, :],
            in1=s0.unsqueeze(2).to_broadcast([P, NTILES, G]),
            op=Alu.is_equal,
        )
        nm0 = wk.tile([P, NTILES, G], f32)
        nc.vector.tensor_scalar(
            nm0[:], eq0[:], 1.0, BIG, op0=Alu.not_equal, op1=Alu.mult
        )
        vpos = wk.tile([P, NTILES, G], f32)
        nc.vector.tensor_tensor(
            out=vpos[:], in0=nm0[:], in1=POS4[:, :, 0, 1 : G + 1], op=Alu.add
        )
        i0p = wk.tile([P, NTILES], f32)
        nc.vector.tensor_reduce(out=i0p[:], in_=vpos[:], axis=Ax.X, op=Alu.min)

        # s1: max candidate over all windows excluding position i0
        nem = wk.tile([P, NTILES * K, G], f32)
        nc.vector.tensor_tensor(
            out=nem[:].rearrange("p (t k) m -> p t k m", k=K),
            in0=POS4[:, :, :, 1 : G + 1],
            in1=i0p[:].unsqueeze(2).unsqueeze(3).to_broadcast([P, NTILES, K, G]),
            op=Alu.not_equal,
        )
        CV2 = wk.tile([P, NTILES * K, G], f32)
        nc.vector.tensor_tensor(out=CV2[:], in0=CV[:], in1=nem[:], op=Alu.mult)
        s1 = wk.tile([P, NTILES], f32)
        nc.vector.tensor_reduce(
            out=s1[:],
            in_=CV2[:].rearrange("p (t k) m -> p t k m", k=K),
            axis=Ax.XY,
            op=Alu.max,
        )

        # neighbors of i0 (from the top-group window, slot 0)
        tm1 = wk.tile([P, NTILES], f32)
        nc.vector.tensor_scalar(
            tm1[:], i0p[:], 1.0, 0.0, op0=Alu.subtract, op1=Alu.max
        )
        tp1 = wk.tile([P, NTILES], f32)
        nc.vector.tensor_scalar(
            tp1[:], i0p[:], 1.0, float(NT - 1), op0=Alu.add, op1=Alu.min
        )

        # results tile R[p, t, 0:5] = (weight, y0, ym1, yp1, i0)
        R = wk.tile([P, NTILES, 5], f32)

        for dst, sel in ((2, tm1), (3, tp1)):
            em = wk.tile([P, NTILES, WIN], f32, tag=f"em{dst}")
            nc.vector.tensor_tensor(
                out=em[:],
                in0=POS4[:, :, 0, :],
                in1=sel[:].unsqueeze(2).to_broadcast([P, NTILES, WIN]),
                op=Alu.is_equal,
            )
            pm = wk.tile([P, NTILES, WIN], f32, tag=f"pm{dst}")
            nc.vector.tensor_tensor(
                out=pm[:],
                in0=em[:],
                in1=AW[:].rearrange("p (t k) j -> p t k j", k=K)[:, :, 0, :],
                op=Alu.mult,
            )
            nc.vector.tensor_reduce(
                out=R[:, :, dst], in_=pm[:], axis=Ax.X, op=Alu.max
            )

        # weight = (0.1 + 3*(s0-s1)) * s0^2
        dd = wk.tile([P, NTILES], f32)
        nc.vector.tensor_tensor(out=dd[:], in0=s0, in1=s1[:], op=Alu.subtract)
        w1 = wk.tile([P, NTILES], f32)
        nc.vector.tensor_scalar(w1[:], dd[:], 3.0, 0.1, op0=Alu.mult, op1=Alu.add)
        s0sq = wk.tile([P, NTILES], f32)
        nc.vector.tensor_tensor(out=s0sq[:], in0=s0, in1=s0, op=Alu.mult)
        nc.vector.tensor_tensor(out=R[:, :, 0], in0=w1[:], in1=s0sq[:], op=Alu.mult)
        nc.vector.tensor_copy(R[:, :, 1], s0)
        nc.vector.tensor_copy(R[:, :, 4], i0p[:])

        # ---- channel combine: slot = c*2 + j ; argmax weight over c ----
        def exact_select(ga, on_true, on_false, name):
            # ga*on_true + (1-ga)*on_false: exact (one factor always 0, other 1)
            ngt = wk.tile([P, 2], f32, tag=f"ng_{name}")
            nc.vector.tensor_scalar(ngt[:], ga[:], 0.5, None, op0=Alu.is_lt)
            gb = ga[:].unsqueeze(2).to_broadcast([P, 2, 5])
            ngb = ngt[:].unsqueeze(2).to_broadcast([P, 2, 5])
            a1 = wk.tile([P, 2, 5], f32, tag=f"a1_{name}")
            nc.vector.tensor_tensor(out=a1[:], in0=on_true, in1=gb, op=Alu.mult)
            a2 = wk.tile([P, 2, 5], f32, tag=f"a2_{name}")
            nc.vector.tensor_tensor(out=a2[:], in0=on_false, in1=ngb, op=Alu.mult)
            res = wk.tile([P, 2, 5], f32, tag=f"res_{name}")
            nc.vector.tensor_tensor(out=res[:], in0=a1[:], in1=a2[:], op=Alu.add)
            return res

        g01 = wk.tile([P, 2], f32)
        nc.vector.tensor_tensor(
            out=g01[:], in0=R[:, 0:2, 0], in1=R[:, 2:4, 0], op=Alu.is_ge
        )
        B01 = exact_select(g01, R[:, 0:2, :], R[:, 2:4, :], "b01")
        g2 = wk.tile([P, 2], f32)
        nc.vector.tensor_tensor(
            out=g2[:], in0=B01[:, :, 0], in1=R[:, 4:6, 0], op=Alu.is_ge
        )
        FIN = exact_select(g2, B01[:], R[:, 4:6, :], "fin")

        # ---- parabola + grid argmax for the winning channel ----
        sm = wk.tile([P, 2], f32)
        nc.vector.tensor_tensor(
            out=sm[:], in0=FIN[:, :, 2], in1=FIN[:, :, 3], op=Alu.add
        )
        acf = wk.tile([P, 2], f32)
        nc.vector.scalar_tensor_tensor(
            out=acf[:],
            in0=sm[:],
            scalar=0.5,
            in1=FIN[:, :, 1],
            op0=Alu.mult,
            op1=Alu.subtract,
        )
        b2 = wk.tile([P, 2], f32)
        nc.vector.tensor_tensor(
            out=b2[:], in0=FIN[:, :, 3], in1=FIN[:, :, 2], op=Alu.subtract
        )
        bcf = wk.tile([P, 2], f32)
        nc.vector.tensor_scalar_mul(bcf[:], b2[:], 0.5)

        xgb = xg[:].unsqueeze(1).to_broadcast([P, 2, NGRID])
        t1 = wk.tile([P, 2, NGRID], f32)
        nc.vector.tensor_tensor(
            out=t1[:],
            in0=xgb,
            in1=acf[:].unsqueeze(2).to_broadcast([P, 2, NGRID]),
            op=Alu.mult,
        )
        nc.vector.tensor_tensor(
            out=t1[:],
            in0=t1[:],
            in1=bcf[:].unsqueeze(2).to_broadcast([P, 2, NGRID]),
            op=Alu.add,
        )
        yg = wk.tile([P, 2, NGRID], f32)
        nc.vector.tensor_tensor(out=yg[:], in0=t1[:], in1=xgb, op=Alu.mult)
        nc.vector.tensor_tensor(
            out=yg[:],
            in0=yg[:],
            in1=FIN[:, :, 1].unsqueeze(2).to_broadcast([P, 2, NGRID]),
            op=Alu.add,
        )

        O = wk.tile([P, 8], f32)  # [max_cc | w | shift_t | shift_idx] x (j0,j1)
        nc.vector.tensor_reduce(
            out=O[:, 0:2], in_=yg[:], axis=Ax.X, op=Alu.max
        )
        nmg = wk.tile([P, 2, NGRID], f32)
        nc.vector.tensor_tensor(
            out=nmg[:],
            in0=yg[:],
            in1=O[:, 0:2].unsqueeze(2).to_broadcast([P, 2, NGRID]),
            op=Alu.not_equal,
        )
        vg = wk.tile([P, 2, NGRID], f32)
        nc.vector.scalar_tensor_tensor(
            out=vg[:],
            in0=nmg[:],
            scalar=BIG,
            in1=xgp3[:].unsqueeze(1).to_broadcast([P, 2, NGRID]),
            op0=Alu.mult,
            op1=Alu.add,
        )
        sub3 = wk.tile([P, 2], f32)
        nc.vector.tensor_reduce(out=sub3[:], in_=vg[:], axis=Ax.X, op=Alu.min)

        nc.vector.tensor_copy(O[:, 2:4], FIN[:, :, 0])  # weight
        sub = wk.tile([P, 2], f32)
        nc.vector.tensor_scalar_sub(sub[:], sub3[:], 3.0)  # sub_shift
        idxw = wk.tile([P, 2], f32)
        nc.vector.tensor_tensor(
            out=idxw[:], in0=FIN[:, :, 4], in1=sub[:], op=Alu.add
        )
        nc.vector.tensor_tensor(
            out=O[:, 6:8],
            in0=idxw[:],
            in1=nlag_t[:].to_broadcast([P, 2]),
            op=Alu.subtract,
        )
        nc.vector.tensor_scalar_mul(O[:, 4:6], O[:, 6:8], 1.0 / 100.0)

        nc.sync.dma_start(
            out=outd[:, :].rearrange("m (j p) -> p m j", p=P),
            in_=O[:].rearrange("p (m j) -> p m j", j=2),
        )

        if debug_outputs:
            dumps = {
                "d_GM": (GM, NTILES * NG),
                "d_M8": (M8, NTILES * 8),
                "d_MI": (MI, NTILES * 8),
                "d_idx": (idxu, NTILES * K),
                "d_W": (W, NTILES * K * WIN),
                "d_AW": (AW, NTILES * K * WIN),
                "d_POS": (POS, NTILES * K * WIN),
                "d_CV": (CV, NTILES * K * G),
                "d_i0p": (i0p, NTILES),
                "d_s1": (s1, NTILES),
                "d_R": (R, NTILES * 5),
                "d_FIN": (FIN, 10),
                "d_sub3": (sub3, 2),
            }
            for name, (tl, fsz) in dumps.items():
                dt_ = tl[:].dtype
                dd = nc.dram_tensor(name, [P, fsz], dt_, kind="ExternalOutput")
                nc.sync.dma_start(
                    out=dd[:, :],
                    in_=tl[:].rearrange("p ... -> p (...)")
                    if tl[:].ndim > 2
                    else tl[:],
                )

    nc.finalize()
    return nc


def _get_nc():
    if "nc" not in _CACHE:
        _CACHE["nc"] = _build_nc()
    return _CACHE["nc"]


def _xg_host():
    import jax
    import jax.numpy as jnp

    with jax.default_device(jax.devices("cpu")[0]):
        return np.asarray(jnp.linspace(-1.0, 1.0, NGRID, dtype=jnp.float32))


def shard_inputs(xcorr, nlag):
    """Full [32,3,64,4096] -> list of 8 per-core input maps."""
    xcorr = np.asarray(xcorr, dtype=np.float32)
    xg = _xg_host()
    nlag_f = np.full([P, 1], float(int(nlag)), dtype=np.float32)
    in_maps = []
    for k in range(NCORES):
        sh = xcorr[k * BPC : (k + 1) * BPC]          # [4, 3, 64, 4096]
        sh = np.ascontiguousarray(sh.transpose(1, 0, 2, 3)).reshape(ROWS, NT)
        pad = np.zeros([ROWS, RPAD], dtype=np.float32)
        pad[:, 1 : NT + 1] = sh
        # window-record table: rec[r*NG + g, :] = pad[r, 16g : 16g+18]
        recs = np.lib.stride_tricks.sliding_window_view(pad, WIN, axis=1)[:, ::G, :]
        recs = np.ascontiguousarray(recs).reshape(ROWS * NG, WIN)
        in_maps.append(
            {
                "xpad": pad,
                "rec": recs,
                "xg": xg.reshape(1, NGRID).copy(),
                "nlag_f": nlag_f.copy(),
            }
        )
    return in_maps


def unshard_outputs(results):
    """list of 8 per-core {'out': [4,256]} -> [4, 32, 1, 64]."""
    full = np.zeros([4, NB, 1, NX], dtype=np.float32)
    for k, res in enumerate(results):
        o = np.asarray(res["out"], dtype=np.float32).reshape(4, BPC, NX)
        full[:, k * BPC : (k + 1) * BPC, 0, :] = o
    return full


def kernel(xcorr, nlag):
    from concourse.bass_utils import run_bass_kernel_spmd

    nc = _get_nc()
    in_maps = shard_inputs(xcorr, nlag)
    res = run_bass_kernel_spmd(nc, in_maps, list(range(NCORES)))
    return unshard_outputs(res.results)
